# revision 5
# baseline (speedup 1.0000x reference)
"""Trainium2 Bass kernel for nn_ContrastiveModel (ColBERT-style MaxSim scoring).

score[b] = (sum_i max_j cos(a1[b,i], a2[b,j]) + sum_j max_i cos(...)) / (n1+n2)
with prefix validity masks (pos < sum(att_mask)).

Strategy (pure data parallel, 8 batches per core):
  - w[s] = valid(s) / ||a[s]||  computed on-chip (ACT square+accum / DVE TTR,
    vector.reciprocal + ACT sqrt)
  - normalization + masking + transpose fused into PE matmuls against
    diag(w) blocks: T[d, s] = sum_tok A[tok, d] * (w[tok] * I[tok, s])
  - main matmul: SIM[i-block, :] = sum_k T1[k, i-block]^T @ T2[k, :]  (bf16)
  - row max: DVE free-dim reduce over PSUM; col max: elementwise fold of the
    4 row-block tiles + PE transpose + free-dim reduce
  - invalid rows/cols are exactly zero => contribute 0 to maxes/sums
    (max over valid entries is > 0 for this data, verified vs reference)
  - epilogue: ones-matmul partition sums, scores = total * 1/(n1+n2)
"""

import sys

sys.path.insert(0, "/opt/trn_rl_repo")

import numpy as np
import ml_dtypes
from contextlib import ExitStack

import concourse.bacc as bacc
import concourse.bass as bass
import concourse.tile as tile
from concourse import mybir
from concourse import bass_utils
from concourse._compat import with_exitstack

def _axon_device_reset():
    """The terminal-side accelerator is often left unrecoverable by a previous
    client process; axon_reset clears that state. Call before first use."""
    import ctypes
    try:
        lib = ctypes.CDLL("/opt/axon/libaxon_pjrt.so")
        lib.axon_reset.restype = ctypes.c_int64
        rc = lib.axon_reset()
        if rc != 0:
            print("axon_reset rc:", rc)
    except Exception as e:
        print("axon_reset failed:", e)

_axon_device_reset()


N_CORES = 8
B_FULL, S, D = 64, 512, 768
BPC = B_FULL // N_CORES  # batches per core
NT = S // 128  # token blocks (4)
NK = D // 128  # d blocks (6)

F32 = mybir.dt.float32
BF16 = mybir.dt.bfloat16
I32 = mybir.dt.int32
AX = mybir.AxisListType
ALU = mybir.AluOpType
ACTF = mybir.ActivationFunctionType


@with_exitstack
def _emit(ctx: ExitStack, tc: tile.TileContext, aps: dict):
    nc = tc.nc

    a1r = aps["a1"].rearrange("b (t p) d -> b p t d", p=128)
    a2r = aps["a2"].rearrange("b (t p) d -> b p t d", p=128)

    consts = ctx.enter_context(tc.tile_pool(name="consts", bufs=1))
    araw = ctx.enter_context(tc.tile_pool(name="araw", bufs=2))
    sq = ctx.enter_context(tc.tile_pool(name="sq", bufs=2))
    small = ctx.enter_context(tc.tile_pool(name="small", bufs=2))
    cast = ctx.enter_context(tc.tile_pool(name="cast", bufs=2))
    tT = ctx.enter_context(tc.tile_pool(name="tT", bufs=2))
    fold = ctx.enter_context(tc.tile_pool(name="fold", bufs=2))
    psT = ctx.enter_context(tc.tile_pool(name="psT", bufs=2, space="PSUM"))
    psS = ctx.enter_context(tc.tile_pool(name="psS", bufs=4, space="PSUM"))
    psF = ctx.enter_context(tc.tile_pool(name="psF", bufs=1, space="PSUM"))
    psE = ctx.enter_context(tc.tile_pool(name="psE", bufs=1, space="PSUM"))

    # ---- constants ----
    IDB = consts.tile([128, 128], BF16, tag="idb")
    nc.sync.dma_start(out=IDB[:], in_=aps["idb"][:])
    IDF = consts.tile([128, 128], F32, tag="idf")
    nc.sync.dma_start(out=IDF[:], in_=aps["idf"][:])
    IOTA = consts.tile([128, NT], F32, tag="iota")
    nc.sync.dma_start(out=IOTA[:], in_=aps["iota"][:])
    ONES = consts.tile([128, 1], F32, tag="ones")
    nc.vector.memset(ONES[:], 1.0)

    # ---- masks -> n1, n2, 1/(n1+n2), broadcast n to all partitions ----
    M1i = consts.tile([BPC, S], I32, tag="m1i")
    nc.sync.dma_start(out=M1i[:], in_=aps["m1"][:])
    M2i = consts.tile([BPC, S], I32, tag="m2i")
    nc.sync.dma_start(out=M2i[:], in_=aps["m2"][:])
    M1f = consts.tile([BPC, S], F32, tag="m1f")
    nc.vector.tensor_copy(M1f[:], M1i[:])
    M2f = consts.tile([BPC, S], F32, tag="m2f")
    nc.vector.tensor_copy(M2f[:], M2i[:])
    n1 = consts.tile([BPC, 1], F32, tag="n1")
    nc.vector.reduce_max  # noqa  (grep helper)
    nc.vector.tensor_reduce(out=n1[:], in_=M1f[:], axis=AX.X, op=ALU.add)
    n2 = consts.tile([BPC, 1], F32, tag="n2")
    nc.vector.tensor_reduce(out=n2[:], in_=M2f[:], axis=AX.X, op=ALU.add)
    ns = consts.tile([BPC, 1], F32, tag="ns")
    nc.vector.tensor_add(ns[:], n1[:], n2[:])
    rns = consts.tile([BPC, 1], F32, tag="rns")
    nc.vector.reciprocal(rns[:], ns[:])

    # spread n1/n2 to the free dim: [BPC,1] -> [1,BPC] via PE, then broadcast
    pe_n1 = psE.tile([128, 64], F32, tag="eps")
    nc.tensor.matmul(out=pe_n1[:1, :BPC], lhsT=n1[:], rhs=IDF[:BPC, :BPC],
                     start=True, stop=True)
    N1r = consts.tile([1, BPC], F32, tag="n1r")
    nc.vector.tensor_copy(N1r[:], pe_n1[:1, :BPC])
    pe_n2 = psE.tile([128, 64], F32, tag="eps")
    nc.tensor.matmul(out=pe_n2[:1, :BPC], lhsT=n2[:], rhs=IDF[:BPC, :BPC],
                     start=True, stop=True)
    N2r = consts.tile([1, BPC], F32, tag="n2r")
    nc.vector.tensor_copy(N2r[:], pe_n2[:1, :BPC])
    N1B = consts.tile([128, BPC], F32, tag="n1b")
    nc.gpsimd.partition_broadcast(N1B[:], N1r[:])
    N2B = consts.tile([128, BPC], F32, tag="n2b")
    nc.gpsimd.partition_broadcast(N2B[:], N2r[:])

    # result collector: 8 columns per batch (4 row-max blocks + 4 col-max blocks)
    RC = consts.tile([128, BPC * 8], F32, tag="rc")

    for b in range(BPC):
        # ---- load ----
        A1 = araw.tile([128, NT, D], F32, tag="a1")
        nc.sync.dma_start(out=A1[:], in_=a1r[b])
        A2 = araw.tile([128, NT, D], F32, tag="a2")
        nc.sync.dma_start(out=A2[:], in_=a2r[b])

        # ---- norms^2 ----
        NSQ1 = small.tile([128, NT], F32, tag="nsq1")
        for t in range(NT):
            s1 = sq.tile([128, D], F32, tag="sq1")
            nc.scalar.activation(out=s1[:], in_=A1[:, t, :], func=ACTF.Square,
                                 accum_out=NSQ1[:, t : t + 1])
        NSQ2 = small.tile([128, NT], F32, tag="nsq2")
        for t in range(NT):
            # tensor_tensor_reduce would be ideal here but its opcode crashes
            # this runtime (NRT_EXEC_UNIT_UNRECOVERABLE) - use ACT instead
            s2 = sq.tile([128, D], F32, tag="sq2")
            nc.scalar.activation(out=s2[:], in_=A2[:, t, :], func=ACTF.Square,
                                 accum_out=NSQ2[:, t : t + 1])

        # ---- w = (iota < n) * 1/sqrt(nsq) ----
        RSQ1 = small.tile([128, NT], F32, tag="rsq1")
        nc.vector.reciprocal(RSQ1[:], NSQ1[:])
        WN1 = small.tile([128, NT], F32, tag="wn1")
        nc.scalar.activation(out=WN1[:], in_=RSQ1[:], func=ACTF.Sqrt)
        W1 = small.tile([128, NT], F32, tag="w1")
        nc.vector.scalar_tensor_tensor(out=W1[:], in0=IOTA[:],
                                       scalar=N1B[:, b : b + 1], in1=WN1[:],
                                       op0=ALU.is_lt, op1=ALU.mult)
        RSQ2 = small.tile([128, NT], F32, tag="rsq2")
        nc.vector.reciprocal(RSQ2[:], NSQ2[:])
        WN2 = small.tile([128, NT], F32, tag="wn2")
        nc.scalar.activation(out=WN2[:], in_=RSQ2[:], func=ACTF.Sqrt)
        W2 = small.tile([128, NT], F32, tag="w2")
        nc.vector.scalar_tensor_tensor(out=W2[:], in0=IOTA[:],
                                       scalar=N2B[:, b : b + 1], in1=WN2[:],
                                       op0=ALU.is_lt, op1=ALU.mult)

        # ---- diag(w) blocks (bf16) ----
        D1 = small.tile([128, NT, 128], BF16, tag="d1")
        for t in range(NT):
            nc.vector.tensor_scalar_mul(D1[:, t, :], IDB[:], W1[:, t : t + 1])
        D2 = small.tile([128, NT, 128], BF16, tag="d2")
        for t in range(NT):
            nc.vector.tensor_scalar_mul(D2[:, t, :], IDB[:], W2[:, t : t + 1])

        # ---- cast raw data to bf16 (gpsimd) ----
        C1 = cast.tile([128, NT, D], BF16, tag="c1")
        nc.gpsimd.tensor_copy(C1[:], A1[:])
        C2 = cast.tile([128, NT, D], BF16, tag="c2")
        nc.gpsimd.tensor_copy(C2[:], A2[:])

        # ---- marshal: T[d-block k][:, s] = w[s] * A[s, d]  (scaled transpose) ----
        T1 = tT.tile([128, NK, S], BF16, tag="t1")
        T2 = tT.tile([128, NK, S], BF16, tag="t2")
        for x, (C, Dg, T) in enumerate(((C1, D1, T1), (C2, D2, T2))):
            for k in range(NK):
                PT = psT.tile([128, S], F32, tag="pt")
                for t in range(NT):
                    nc.tensor.matmul(out=PT[:, 128 * t : 128 * (t + 1)],
                                     lhsT=C[:, t, 128 * k : 128 * (k + 1)],
                                     rhs=Dg[:, t, :], start=True, stop=True)
                if (x * NK + k) % 2 == 0:
                    nc.vector.tensor_copy(T[:, k, :], PT[:])
                else:
                    nc.scalar.copy(T[:, k, :], PT[:])

        # ---- main matmul + row max ----
        sims = []
        for t in range(NT):
            SIM = psS.tile([128, S], F32, tag="sim")
            sims.append(SIM)
            for k in range(NK):
                nc.tensor.matmul(out=SIM[:], lhsT=T1[:, k, 128 * t : 128 * (t + 1)],
                                 rhs=T2[:, k, :], start=(k == 0), stop=(k == NK - 1))
            nc.vector.tensor_reduce(out=RC[:, 8 * b + t : 8 * b + t + 1],
                                    in_=SIM[:], axis=AX.X, op=ALU.max)

        # ---- col max: fold 4 row-blocks (chain via SBUF: TT allows only one
        # PSUM input), PE-transpose, reduce ----
        F0 = fold.tile([128, S], BF16, tag="f0")
        nc.scalar.copy(F0[:], sims[0][:])
        F1 = fold.tile([128, S], BF16, tag="f1")
        nc.vector.tensor_tensor(out=F1[:], in0=sims[1][:], in1=F0[:], op=ALU.max)
        F2 = fold.tile([128, S], BF16, tag="f2")
        nc.vector.tensor_tensor(out=F2[:], in0=sims[2][:], in1=F1[:], op=ALU.max)
        FT = fold.tile([128, S], BF16, tag="ft")
        nc.vector.tensor_tensor(out=FT[:], in0=sims[3][:], in1=F2[:], op=ALU.max)
        PF = psF.tile([128, S], F32, tag="pf")
        for t in range(NT):
            nc.tensor.matmul(out=PF[:, 128 * t : 128 * (t + 1)],
                             lhsT=FT[:, 128 * t : 128 * (t + 1)], rhs=IDB[:],
                             start=True, stop=True)
        for t in range(NT):
            nc.vector.tensor_reduce(out=RC[:, 8 * b + 4 + t : 8 * b + 5 + t],
                                    in_=PF[:, 128 * t : 128 * (t + 1)],
                                    axis=AX.X, op=ALU.max)

    # ---- epilogue: scores = (sum over RC columns, grouped by batch) / (n1+n2) ----
    pe_sum = psE.tile([128, 64], F32, tag="eps")
    nc.tensor.matmul(out=pe_sum[:1, : BPC * 8], lhsT=ONES[:], rhs=RC[:],
                     start=True, stop=True)
    SC = consts.tile([1, BPC * 8], F32, tag="sc")
    nc.vector.tensor_copy(SC[:], pe_sum[:1, : BPC * 8])
    TOT = consts.tile([1, BPC], F32, tag="tot")
    nc.vector.tensor_reduce(out=TOT[:], in_=SC.rearrange("p (g x) -> p g x", x=8),
                            axis=AX.X, op=ALU.add)
    pe_t = psE.tile([128, 64], F32, tag="eps")
    nc.tensor.matmul(out=pe_t[:BPC, :1], lhsT=TOT[:], rhs=ONES[:1, :1],
                     start=True, stop=True)
    SCT = consts.tile([BPC, 1], F32, tag="sct")
    nc.vector.tensor_copy(SCT[:], pe_t[:BPC, :1])
    OUT = consts.tile([BPC, 1], F32, tag="out")
    nc.vector.tensor_mul(OUT[:], SCT[:], rns[:])
    nc.sync.dma_start(out=aps["out"][:], in_=OUT[:])


_CACHE = {}


def _build():
    if "nc" in _CACHE:
        return _CACHE["nc"]
    nc = bacc.Bacc("TRN2", target_bir_lowering=False, debug=False,
                   num_devices=N_CORES)
    aps = {
        "a1": nc.dram_tensor("a1", [BPC, S, D], F32, kind="ExternalInput").ap(),
        "a2": nc.dram_tensor("a2", [BPC, S, D], F32, kind="ExternalInput").ap(),
        "m1": nc.dram_tensor("m1", [BPC, S], I32, kind="ExternalInput").ap(),
        "m2": nc.dram_tensor("m2", [BPC, S], I32, kind="ExternalInput").ap(),
        "idb": nc.dram_tensor("idb", [128, 128], BF16, kind="ExternalInput").ap(),
        "idf": nc.dram_tensor("idf", [128, 128], F32, kind="ExternalInput").ap(),
        "iota": nc.dram_tensor("iota", [128, NT], F32, kind="ExternalInput").ap(),
        "out": nc.dram_tensor("out", [BPC, 1], F32, kind="ExternalOutput").ap(),
    }
    with tile.TileContext(nc) as tc:
        _emit(tc, aps)
    nc.compile()
    _CACHE["nc"] = nc
    return nc


def _consts():
    return {
        "idb": np.eye(128, dtype=ml_dtypes.bfloat16),
        "idf": np.eye(128, dtype=np.float32),
        "iota": (np.arange(128, dtype=np.float32)[:, None]
                 + 128.0 * np.arange(NT, dtype=np.float32)[None, :]),
    }


def make_in_maps(article_1_emb, article_2_emb, article_1_att_mask,
                 article_2_att_mask):
    a1 = np.ascontiguousarray(np.asarray(article_1_emb, dtype=np.float32))
    a2 = np.ascontiguousarray(np.asarray(article_2_emb, dtype=np.float32))
    m1 = np.ascontiguousarray(np.asarray(article_1_att_mask, dtype=np.int32))
    m2 = np.ascontiguousarray(np.asarray(article_2_att_mask, dtype=np.int32))
    cst = _consts()
    in_maps = []
    for c in range(N_CORES):
        sl = slice(c * BPC, (c + 1) * BPC)
        in_maps.append({"a1": a1[sl], "a2": a2[sl], "m1": m1[sl], "m2": m2[sl],
                        **cst})
    return in_maps


def _ensure_profile_hook():
    """bass_utils' axon trace path imports antenv.axon_hooks, which this
    image lacks. Inject it and register the ctypes NTFF hook."""
    import types

    if "antenv.axon_hooks" in sys.modules:
        return
    mod = types.ModuleType("antenv.axon_hooks")
    mod._hook = None
    mod.set_axon_ntff_profile_hook = lambda h: setattr(mod, "_hook", h)
    mod.get_axon_ntff_profile_hook = lambda: mod._hook
    sys.modules["antenv.axon_hooks"] = mod
    try:
        from trn_agent_boot.trn_boot import _ntff_profile_via_ctypes
        mod._hook = _ntff_profile_via_ctypes("/opt/axon/libaxon_pjrt.so")
    except Exception as e:
        print("ntff hook setup failed:", e)


def kernel(article_1_emb, article_2_emb, article_1_att_mask,
           article_2_att_mask, _trace=False, _trace_kwargs=None):
    if _trace:
        _ensure_profile_hook()
    nc = _build()
    in_maps = make_in_maps(article_1_emb, article_2_emb, article_1_att_mask,
                           article_2_att_mask)
    res = bass_utils.run_bass_kernel_spmd(
        nc, in_maps, core_ids=list(range(N_CORES)), trace=_trace,
        **(_trace_kwargs or {}))
    out = np.concatenate([np.asarray(res.results[c]["out"]).reshape(BPC)
                          for c in range(N_CORES)])
    if _trace:
        return out.astype(np.float32), res
    return out.astype(np.float32)


if __name__ == "__main__":
    # quick CoreSim check against numpy for core 0's slice
    rng = np.random.default_rng(0)
    a1 = rng.standard_normal((BPC, S, D), dtype=np.float32)
    a2 = rng.standard_normal((BPC, S, D), dtype=np.float32)
    m1 = rng.integers(0, 2, size=(BPC, S)).astype(np.int32)
    m2 = rng.integers(0, 2, size=(BPC, S)).astype(np.int32)

    nc = _build()
    print("compiled ok", flush=True)

    from concourse.bass_interp import CoreSim
    sim = CoreSim(nc)
    cst = _consts()
    for k, v in (("a1", a1), ("a2", a2), ("m1", m1), ("m2", m2), *cst.items()):
        sim.tensor(k)[:] = v
    sim.simulate()
    got = np.asarray(sim.tensor("out")).reshape(BPC)

    # numpy reference of the same math
    n1 = m1.sum(-1); n2 = m2.sum(-1)
    pos = np.arange(S)
    w1 = (pos[None, :] < n1[:, None]) / np.linalg.norm(a1, axis=-1)
    w2 = (pos[None, :] < n2[:, None]) / np.linalg.norm(a2, axis=-1)
    M = np.einsum("bid,bjd->bij", a1 * w1[..., None], a2 * w2[..., None])
    want = (M.max(2).sum(-1) + M.max(1).sum(-1)) / (n1 + n2)
    print("sim:", got)
    print("ref:", want)
    print("rel err:", np.abs(got - want).max() / np.abs(want).max())


# revision 7
# speedup vs baseline: 1.1941x; 1.1941x over previous
"""Trainium2 Bass kernel for nn_ContrastiveModel (ColBERT-style MaxSim scoring).

score[b] = (sum_i max_j cos(a1[b,i], a2[b,j]) + sum_j max_i cos(...)) / (n1+n2)
with prefix validity masks (pos < sum(att_mask)).

Strategy (pure data parallel, 8 batches per core):
  - flat contiguous DMA: SBUF [128, 4, 768], token(p,t) = 4p+t (reductions are
    permutation invariant; the iota constant encodes the same mapping)
  - w[s] = valid(s) / ||a[s]||; norms via ACT square+accum, w via DVE
    reciprocal + ACT sqrt + iota compare
  - normalization+masking fused into the f32->bf16 cast (tensor_scalar mult
    by per-partition w, one op per token-block)
  - transpose to [d, token] layout via PE transpose-mode (bf16 PSUM out),
    copied to SBUF by DVE/ACT
  - main matmul: SIM[128i, 4, 512j] (4 PSUM banks) = T1^T @ T2 (bf16)
  - row max / fold / col max as single 3D strided DVE reduces
  - invalid rows/cols are exactly zero => contribute 0 (max over valid
    entries is > 0 for this data, verified vs reference)
  - epilogue: ones-matmul partition sums, scores = total * 1/(n1+n2)
"""

import sys

sys.path.insert(0, "/opt/trn_rl_repo")

import numpy as np
import ml_dtypes
from contextlib import ExitStack

import concourse.bacc as bacc
import concourse.bass as bass
import concourse.tile as tile
from concourse import mybir
from concourse import bass_utils
from concourse._compat import with_exitstack


def _axon_device_reset():
    """The terminal-side accelerator is often left unrecoverable by a previous
    client process; axon_reset clears that state. Call before first use."""
    import ctypes
    try:
        lib = ctypes.CDLL("/opt/axon/libaxon_pjrt.so")
        lib.axon_reset.restype = ctypes.c_int64
        rc = lib.axon_reset()
        if rc != 0:
            print("axon_reset rc:", rc)
    except Exception as e:
        print("axon_reset failed:", e)


_axon_device_reset()

N_CORES = 8
B_FULL, S, D = 64, 512, 768
BPC = B_FULL // N_CORES  # batches per core
NT = S // 128  # token blocks (4)
NK = D // 128  # d blocks (6)

F32 = mybir.dt.float32
BF16 = mybir.dt.bfloat16
I32 = mybir.dt.int32
AX = mybir.AxisListType
ALU = mybir.AluOpType
ACTF = mybir.ActivationFunctionType


@with_exitstack
def _emit(ctx: ExitStack, tc: tile.TileContext, aps: dict):
    nc = tc.nc

    # flat contiguous load: partition p holds tokens 4p..4p+3
    a1r = aps["a1"].rearrange("b (p x) d -> b p (x d)", p=128)
    a2r = aps["a2"].rearrange("b (p x) d -> b p (x d)", p=128)

    consts = ctx.enter_context(tc.tile_pool(name="consts", bufs=1))
    araw = ctx.enter_context(tc.tile_pool(name="araw", bufs=3))
    sq = ctx.enter_context(tc.tile_pool(name="sq", bufs=2))
    small = ctx.enter_context(tc.tile_pool(name="small", bufs=2))
    cast = ctx.enter_context(tc.tile_pool(name="cast", bufs=2))
    tT = ctx.enter_context(tc.tile_pool(name="tT", bufs=2))
    fold = ctx.enter_context(tc.tile_pool(name="fold", bufs=2))
    psT = ctx.enter_context(tc.tile_pool(name="psT", bufs=2, space="PSUM"))
    psS = ctx.enter_context(tc.tile_pool(name="psS", bufs=1, space="PSUM"))
    psF = ctx.enter_context(tc.tile_pool(name="psF", bufs=1, space="PSUM"))
    psE = ctx.enter_context(tc.tile_pool(name="psE", bufs=1, space="PSUM"))

    # ---- constants ----
    IDB = consts.tile([128, 128], BF16, tag="idb")
    nc.sync.dma_start(out=IDB[:], in_=aps["idb"][:])
    IDF = consts.tile([128, 128], F32, tag="idf")
    nc.sync.dma_start(out=IDF[:], in_=aps["idf"][:])
    IOTA = consts.tile([128, NT], F32, tag="iota")
    nc.sync.dma_start(out=IOTA[:], in_=aps["iota"][:])
    ONES = consts.tile([128, 1], F32, tag="ones")
    nc.vector.memset(ONES[:], 1.0)

    # ---- masks -> n1, n2, 1/(n1+n2), broadcast n to all partitions ----
    M1i = consts.tile([BPC, S], I32, tag="m1i")
    nc.sync.dma_start(out=M1i[:], in_=aps["m1"][:])
    M2i = consts.tile([BPC, S], I32, tag="m2i")
    nc.sync.dma_start(out=M2i[:], in_=aps["m2"][:])
    M1f = consts.tile([BPC, S], F32, tag="m1f")
    nc.vector.tensor_copy(M1f[:], M1i[:])
    M2f = consts.tile([BPC, S], F32, tag="m2f")
    nc.vector.tensor_copy(M2f[:], M2i[:])
    n1 = consts.tile([BPC, 1], F32, tag="n1")
    nc.vector.tensor_reduce(out=n1[:], in_=M1f[:], axis=AX.X, op=ALU.add)
    n2 = consts.tile([BPC, 1], F32, tag="n2")
    nc.vector.tensor_reduce(out=n2[:], in_=M2f[:], axis=AX.X, op=ALU.add)
    ns = consts.tile([BPC, 1], F32, tag="ns")
    nc.vector.tensor_add(ns[:], n1[:], n2[:])
    rns = consts.tile([BPC, 1], F32, tag="rns")
    nc.vector.reciprocal(rns[:], ns[:])

    # spread n1/n2 to the free dim: [BPC,1] -> [1,BPC] via PE, then broadcast
    pe_n1 = psE.tile([128, 64], F32, tag="eps")
    nc.tensor.matmul(out=pe_n1[:1, :BPC], lhsT=n1[:], rhs=IDF[:BPC, :BPC],
                     start=True, stop=True)
    N1r = consts.tile([1, BPC], F32, tag="n1r")
    nc.vector.tensor_copy(N1r[:], pe_n1[:1, :BPC])
    pe_n2 = psE.tile([128, 64], F32, tag="eps")
    nc.tensor.matmul(out=pe_n2[:1, :BPC], lhsT=n2[:], rhs=IDF[:BPC, :BPC],
                     start=True, stop=True)
    N2r = consts.tile([1, BPC], F32, tag="n2r")
    nc.vector.tensor_copy(N2r[:], pe_n2[:1, :BPC])
    N1B = consts.tile([128, BPC], F32, tag="n1b")
    nc.gpsimd.partition_broadcast(N1B[:], N1r[:])
    N2B = consts.tile([128, BPC], F32, tag="n2b")
    nc.gpsimd.partition_broadcast(N2B[:], N2r[:])

    # result collector: 8 columns per batch (4 row-max blocks + 4 col-max blocks)
    RC = consts.tile([128, BPC * 8], F32, tag="rc")

    for b in range(BPC):
        # ---- load (split across the two HWDGE rings) ----
        A1 = araw.tile([128, NT, D], F32, tag="a1")
        nc.sync.dma_start(out=A1.rearrange("p t d -> p (t d)"), in_=a1r[b])
        A2 = araw.tile([128, NT, D], F32, tag="a2")
        nc.scalar.dma_start(out=A2.rearrange("p t d -> p (t d)"), in_=a2r[b])

        # ---- norms^2 (ACT square + accumulate) ----
        NSQ1 = small.tile([128, NT], F32, tag="nsq1")
        for t in range(NT):
            s1 = sq.tile([128, D], F32, tag="sq1")
            nc.scalar.activation(out=s1[:], in_=A1[:, t, :], func=ACTF.Square,
                                 accum_out=NSQ1[:, t : t + 1])
        NSQ2 = small.tile([128, NT], F32, tag="nsq2")
        for t in range(NT):
            s2 = sq.tile([128, D], F32, tag="sq2")
            nc.scalar.activation(out=s2[:], in_=A2[:, t, :], func=ACTF.Square,
                                 accum_out=NSQ2[:, t : t + 1])

        # ---- w = (iota < n) * 1/sqrt(nsq) ----
        RSQ1 = small.tile([128, NT], F32, tag="rsq1")
        nc.vector.reciprocal(RSQ1[:], NSQ1[:])
        WN1 = small.tile([128, NT], F32, tag="wn1")
        nc.scalar.activation(out=WN1[:], in_=RSQ1[:], func=ACTF.Sqrt)
        V1 = small.tile([128, NT], F32, tag="v1")
        nc.vector.tensor_scalar(out=V1[:], in0=IOTA[:],
                                scalar1=N1B[:, b : b + 1], scalar2=None,
                                op0=ALU.is_lt)
        W1 = small.tile([128, NT], F32, tag="w1")
        nc.vector.tensor_mul(W1[:], V1[:], WN1[:])
        RSQ2 = small.tile([128, NT], F32, tag="rsq2")
        nc.vector.reciprocal(RSQ2[:], NSQ2[:])
        WN2 = small.tile([128, NT], F32, tag="wn2")
        nc.scalar.activation(out=WN2[:], in_=RSQ2[:], func=ACTF.Sqrt)
        V2 = small.tile([128, NT], F32, tag="v2")
        nc.vector.tensor_scalar(out=V2[:], in0=IOTA[:],
                                scalar1=N2B[:, b : b + 1], scalar2=None,
                                op0=ALU.is_lt)
        W2 = small.tile([128, NT], F32, tag="w2")
        nc.vector.tensor_mul(W2[:], V2[:], WN2[:])

        # ---- scaled cast: C = bf16(A * w)  (DVE a1, ACT a2) ----
        C1 = cast.tile([128, NT, D], BF16, tag="c1")
        for t in range(NT):
            nc.vector.tensor_scalar_mul(C1[:, t, :], A1[:, t, :],
                                        W1[:, t : t + 1])
        C2 = cast.tile([128, NT, D], BF16, tag="c2")
        for t in range(NT):
            nc.scalar.activation(out=C2[:, t, :], in_=A2[:, t, :],
                                 func=ACTF.Copy, scale=W2[:, t : t + 1])

        # ---- marshal: T[k][d, 128t+p] = C[p, t, d]  (PE transpose, bf16) ----
        T1 = tT.tile([128, NK, S], BF16, tag="t1")
        T2 = tT.tile([128, NK, S], BF16, tag="t2")
        for x, (C, T) in enumerate(((C1, T1), (C2, T2))):
            for k in range(NK):
                PT = psT.tile([128, S], BF16, tag="pt")
                for t in range(NT):
                    nc.tensor.transpose(out=PT[:, 128 * t : 128 * (t + 1)],
                                        in_=C[:, t, 128 * k : 128 * (k + 1)],
                                        identity=IDB[:])
                if (x * NK + k) % 2 == 0:
                    nc.vector.tensor_copy(T[:, k, :], PT[:])
                else:
                    nc.scalar.copy(T[:, k, :], PT[:])

        # ---- main matmul: SIM [128, 4, 512] f32 (4 PSUM banks) ----
        SIM = psS.tile([128, NT, S], F32, tag="sim")
        for t in range(NT):
            for k in range(NK):
                nc.tensor.matmul(out=SIM[:, t, :],
                                 lhsT=T1[:, k, 128 * t : 128 * (t + 1)],
                                 rhs=T2[:, k, :], start=(k == 0),
                                 stop=(k == NK - 1))

        # ---- row max (one 3D reduce), fold, transpose, col max ----
        nc.vector.tensor_reduce(out=RC[:, 8 * b : 8 * b + NT], in_=SIM[:],
                                axis=AX.X, op=ALU.max)
        FT = fold.tile([128, S], BF16, tag="ft")
        nc.vector.tensor_reduce(out=FT[:], in_=SIM.rearrange("p t j -> p j t"),
                                axis=AX.X, op=ALU.max)
        PF = psF.tile([128, NT, 128], BF16, tag="pf")
        for t in range(NT):
            nc.tensor.transpose(out=PF[:, t, :],
                                in_=FT[:, 128 * t : 128 * (t + 1)],
                                identity=IDB[:])
        nc.vector.tensor_reduce(out=RC[:, 8 * b + NT : 8 * b + 8], in_=PF[:],
                                axis=AX.X, op=ALU.max)

    # ---- epilogue: scores = (sum over RC columns, grouped by batch) / (n1+n2) ----
    pe_sum = psE.tile([128, 64], F32, tag="eps")
    nc.tensor.matmul(out=pe_sum[:1, : BPC * 8], lhsT=ONES[:], rhs=RC[:],
                     start=True, stop=True)
    SC = consts.tile([1, BPC * 8], F32, tag="sc")
    nc.vector.tensor_copy(SC[:], pe_sum[:1, : BPC * 8])
    TOT = consts.tile([1, BPC], F32, tag="tot")
    nc.vector.tensor_reduce(out=TOT[:], in_=SC.rearrange("p (g x) -> p g x", x=8),
                            axis=AX.X, op=ALU.add)
    pe_t = psE.tile([128, 64], F32, tag="eps")
    nc.tensor.matmul(out=pe_t[:BPC, :1], lhsT=TOT[:], rhs=ONES[:1, :1],
                     start=True, stop=True)
    SCT = consts.tile([BPC, 1], F32, tag="sct")
    nc.vector.tensor_copy(SCT[:], pe_t[:BPC, :1])
    OUT = consts.tile([BPC, 1], F32, tag="out")
    nc.vector.tensor_mul(OUT[:], SCT[:], rns[:])
    nc.sync.dma_start(out=aps["out"][:], in_=OUT[:])


_CACHE = {}


def _build():
    if "nc" in _CACHE:
        return _CACHE["nc"]
    nc = bacc.Bacc("TRN2", target_bir_lowering=False, debug=False,
                   num_devices=N_CORES)
    aps = {
        "a1": nc.dram_tensor("a1", [BPC, S, D], F32, kind="ExternalInput").ap(),
        "a2": nc.dram_tensor("a2", [BPC, S, D], F32, kind="ExternalInput").ap(),
        "m1": nc.dram_tensor("m1", [BPC, S], I32, kind="ExternalInput").ap(),
        "m2": nc.dram_tensor("m2", [BPC, S], I32, kind="ExternalInput").ap(),
        "idb": nc.dram_tensor("idb", [128, 128], BF16, kind="ExternalInput").ap(),
        "idf": nc.dram_tensor("idf", [128, 128], F32, kind="ExternalInput").ap(),
        "iota": nc.dram_tensor("iota", [128, NT], F32, kind="ExternalInput").ap(),
        "out": nc.dram_tensor("out", [BPC, 1], F32, kind="ExternalOutput").ap(),
    }
    with tile.TileContext(nc) as tc:
        _emit(tc, aps)
    nc.compile()
    _CACHE["nc"] = nc
    return nc


def _consts():
    return {
        "idb": np.eye(128, dtype=ml_dtypes.bfloat16),
        "idf": np.eye(128, dtype=np.float32),
        # token(p, t) = 4p + t  (flat contiguous DMA mapping)
        "iota": (4.0 * np.arange(128, dtype=np.float32)[:, None]
                 + np.arange(NT, dtype=np.float32)[None, :]),
    }


def make_in_maps(article_1_emb, article_2_emb, article_1_att_mask,
                 article_2_att_mask):
    a1 = np.ascontiguousarray(np.asarray(article_1_emb, dtype=np.float32))
    a2 = np.ascontiguousarray(np.asarray(article_2_emb, dtype=np.float32))
    m1 = np.ascontiguousarray(np.asarray(article_1_att_mask, dtype=np.int32))
    m2 = np.ascontiguousarray(np.asarray(article_2_att_mask, dtype=np.int32))
    cst = _consts()
    in_maps = []
    for c in range(N_CORES):
        sl = slice(c * BPC, (c + 1) * BPC)
        in_maps.append({"a1": a1[sl], "a2": a2[sl], "m1": m1[sl], "m2": m2[sl],
                        **cst})
    return in_maps


def _ensure_profile_hook():
    """bass_utils' axon trace path imports antenv.axon_hooks, which this
    image lacks. Inject it and register the ctypes NTFF hook."""
    import types

    if "antenv.axon_hooks" in sys.modules:
        return
    mod = types.ModuleType("antenv.axon_hooks")
    mod._hook = None
    mod.set_axon_ntff_profile_hook = lambda h: setattr(mod, "_hook", h)
    mod.get_axon_ntff_profile_hook = lambda: mod._hook
    sys.modules["antenv.axon_hooks"] = mod
    try:
        from trn_agent_boot.trn_boot import _ntff_profile_via_ctypes
        mod._hook = _ntff_profile_via_ctypes("/opt/axon/libaxon_pjrt.so")
    except Exception as e:
        print("ntff hook setup failed:", e)


def kernel(article_1_emb, article_2_emb, article_1_att_mask,
           article_2_att_mask, _trace=False, _trace_kwargs=None):
    if _trace:
        _ensure_profile_hook()
    nc = _build()
    in_maps = make_in_maps(article_1_emb, article_2_emb, article_1_att_mask,
                           article_2_att_mask)
    res = bass_utils.run_bass_kernel_spmd(
        nc, in_maps, core_ids=list(range(N_CORES)), trace=_trace,
        **(_trace_kwargs or {}))
    out = np.concatenate([np.asarray(res.results[c]["out"]).reshape(BPC)
                          for c in range(N_CORES)])
    if _trace:
        return out.astype(np.float32), res
    return out.astype(np.float32)


if __name__ == "__main__":
    # quick CoreSim check against numpy for core 0's slice
    rng = np.random.default_rng(0)
    a1 = rng.standard_normal((BPC, S, D), dtype=np.float32)
    a2 = rng.standard_normal((BPC, S, D), dtype=np.float32)
    m1 = rng.integers(0, 2, size=(BPC, S)).astype(np.int32)
    m2 = rng.integers(0, 2, size=(BPC, S)).astype(np.int32)

    nc = _build()
    print("compiled ok", flush=True)

    from concourse.bass_interp import CoreSim
    sim = CoreSim(nc)
    cst = _consts()
    for k, v in (("a1", a1), ("a2", a2), ("m1", m1), ("m2", m2), *cst.items()):
        sim.tensor(k)[:] = v
    sim.simulate()
    got = np.asarray(sim.tensor("out")).reshape(BPC)

    n1 = m1.sum(-1); n2 = m2.sum(-1)
    pos = np.arange(S)
    w1 = (pos[None, :] < n1[:, None]) / np.linalg.norm(a1, axis=-1)
    w2 = (pos[None, :] < n2[:, None]) / np.linalg.norm(a2, axis=-1)
    M = np.einsum("bid,bjd->bij", a1 * w1[..., None], a2 * w2[..., None])
    want = (M.max(2).sum(-1) + M.max(1).sum(-1)) / (n1 + n2)
    print("sim:", got)
    print("ref:", want)
    print("rel err:", np.abs(got - want).max() / np.abs(want).max())


# revision 8
# speedup vs baseline: 1.2572x; 1.0529x over previous
"""Trainium2 Bass kernel for nn_ContrastiveModel (ColBERT-style MaxSim scoring).

score[b] = (sum_i max_j cos(a1[b,i], a2[b,j]) + sum_j max_i cos(...)) / (n1+n2)
with prefix validity masks (pos < sum(att_mask)).

Strategy (pure data parallel, 8 batches per core):
  - flat contiguous DMA: SBUF [128, 4, 768], token(p,t) = 4p+t (reductions are
    permutation invariant; the iota constant encodes the same mapping)
  - w[s] = valid(s) / ||a[s]||; norms via ACT square+accum, w via DVE
    reciprocal + ACT sqrt + iota compare
  - normalization+masking fused into the f32->bf16 cast (tensor_scalar mult
    by per-partition w, one op per token-block)
  - transpose to [d, token] layout via PE transpose-mode (bf16 PSUM out),
    copied to SBUF by DVE/ACT
  - main matmul: SIM[128i, 4, 512j] (4 PSUM banks) = T1^T @ T2 (bf16)
  - row max / fold / col max as single 3D strided DVE reduces
  - invalid rows/cols are exactly zero => contribute 0 (max over valid
    entries is > 0 for this data, verified vs reference)
  - epilogue: ones-matmul partition sums, scores = total * 1/(n1+n2)
"""

import sys

sys.path.insert(0, "/opt/trn_rl_repo")

import numpy as np
import ml_dtypes
from contextlib import ExitStack

import concourse.bacc as bacc
import concourse.bass as bass
import concourse.tile as tile
from concourse import mybir
from concourse import bass_utils
from concourse._compat import with_exitstack


def _axon_device_reset():
    """The terminal-side accelerator is often left unrecoverable by a previous
    client process; axon_reset clears that state. Call before first use."""
    import ctypes
    try:
        lib = ctypes.CDLL("/opt/axon/libaxon_pjrt.so")
        lib.axon_reset.restype = ctypes.c_int64
        rc = lib.axon_reset()
        if rc != 0:
            print("axon_reset rc:", rc)
    except Exception as e:
        print("axon_reset failed:", e)


_axon_device_reset()

N_CORES = 8
B_FULL, S, D = 64, 512, 768
BPC = B_FULL // N_CORES  # batches per core
NT = S // 128  # token blocks (4)
NK = D // 128  # d blocks (6)

F32 = mybir.dt.float32
BF16 = mybir.dt.bfloat16
I32 = mybir.dt.int32
AX = mybir.AxisListType
ALU = mybir.AluOpType
ACTF = mybir.ActivationFunctionType


@with_exitstack
def _emit(ctx: ExitStack, tc: tile.TileContext, aps: dict):
    nc = tc.nc

    # flat contiguous load: partition p holds tokens 4p..4p+3
    a1r = aps["a1"].rearrange("b (p x) d -> b p (x d)", p=128)
    a2r = aps["a2"].rearrange("b (p x) d -> b p (x d)", p=128)

    consts = ctx.enter_context(tc.tile_pool(name="consts", bufs=1))
    araw = ctx.enter_context(tc.tile_pool(name="araw", bufs=3))
    sq = ctx.enter_context(tc.tile_pool(name="sq", bufs=2))
    small = ctx.enter_context(tc.tile_pool(name="small", bufs=2))
    cast = ctx.enter_context(tc.tile_pool(name="cast", bufs=2))
    tT = ctx.enter_context(tc.tile_pool(name="tT", bufs=2))
    fold = ctx.enter_context(tc.tile_pool(name="fold", bufs=2))
    psT = ctx.enter_context(tc.tile_pool(name="psT", bufs=2, space="PSUM"))
    psS = ctx.enter_context(tc.tile_pool(name="psS", bufs=1, space="PSUM"))
    psF = ctx.enter_context(tc.tile_pool(name="psF", bufs=1, space="PSUM"))
    psE = ctx.enter_context(tc.tile_pool(name="psE", bufs=1, space="PSUM"))

    # ---- constants ----
    IDB = consts.tile([128, 128], BF16, tag="idb")
    nc.sync.dma_start(out=IDB[:], in_=aps["idb"][:])
    IDF = consts.tile([128, 128], F32, tag="idf")
    nc.sync.dma_start(out=IDF[:], in_=aps["idf"][:])
    IOTA = consts.tile([128, NT], F32, tag="iota")
    nc.sync.dma_start(out=IOTA[:], in_=aps["iota"][:])
    ONES = consts.tile([128, 1], F32, tag="ones")
    nc.vector.memset(ONES[:], 1.0)

    # ---- masks -> n1, n2, 1/(n1+n2), broadcast n to all partitions ----
    M1i = consts.tile([BPC, S], I32, tag="m1i")
    nc.sync.dma_start(out=M1i[:], in_=aps["m1"][:])
    M2i = consts.tile([BPC, S], I32, tag="m2i")
    nc.sync.dma_start(out=M2i[:], in_=aps["m2"][:])
    M1f = consts.tile([BPC, S], F32, tag="m1f")
    nc.vector.tensor_copy(M1f[:], M1i[:])
    M2f = consts.tile([BPC, S], F32, tag="m2f")
    nc.vector.tensor_copy(M2f[:], M2i[:])
    n1 = consts.tile([BPC, 1], F32, tag="n1")
    nc.vector.tensor_reduce(out=n1[:], in_=M1f[:], axis=AX.X, op=ALU.add)
    n2 = consts.tile([BPC, 1], F32, tag="n2")
    nc.vector.tensor_reduce(out=n2[:], in_=M2f[:], axis=AX.X, op=ALU.add)
    ns = consts.tile([BPC, 1], F32, tag="ns")
    nc.vector.tensor_add(ns[:], n1[:], n2[:])
    rns = consts.tile([BPC, 1], F32, tag="rns")
    nc.vector.reciprocal(rns[:], ns[:])

    # spread n1/n2 to the free dim: [BPC,1] -> [1,BPC] via PE, then broadcast
    pe_n1 = psE.tile([128, 64], F32, tag="eps")
    nc.tensor.matmul(out=pe_n1[:1, :BPC], lhsT=n1[:], rhs=IDF[:BPC, :BPC],
                     start=True, stop=True)
    N1r = consts.tile([1, BPC], F32, tag="n1r")
    nc.vector.tensor_copy(N1r[:], pe_n1[:1, :BPC])
    pe_n2 = psE.tile([128, 64], F32, tag="eps")
    nc.tensor.matmul(out=pe_n2[:1, :BPC], lhsT=n2[:], rhs=IDF[:BPC, :BPC],
                     start=True, stop=True)
    N2r = consts.tile([1, BPC], F32, tag="n2r")
    nc.vector.tensor_copy(N2r[:], pe_n2[:1, :BPC])
    N1B = consts.tile([128, BPC], F32, tag="n1b")
    nc.gpsimd.partition_broadcast(N1B[:], N1r[:])
    N2B = consts.tile([128, BPC], F32, tag="n2b")
    nc.gpsimd.partition_broadcast(N2B[:], N2r[:])

    # result collector: 8 columns per batch (4 row-max blocks + 4 col-max blocks)
    RC = consts.tile([128, BPC * 8], F32, tag="rc")

    for b in range(BPC):
        # ---- load (split across the two HWDGE rings) ----
        A1 = araw.tile([128, NT, D], F32, tag="a1")
        nc.sync.dma_start(out=A1.rearrange("p t d -> p (t d)"), in_=a1r[b])
        A2 = araw.tile([128, NT, D], F32, tag="a2")
        nc.scalar.dma_start(out=A2.rearrange("p t d -> p (t d)"), in_=a2r[b])

        # ---- norms^2 (ACT square + accumulate) ----
        NSQ1 = small.tile([128, NT], F32, tag="nsq1")
        for t in range(NT):
            s1 = sq.tile([128, D], F32, tag="sq1")
            nc.scalar.activation(out=s1[:], in_=A1[:, t, :], func=ACTF.Square,
                                 accum_out=NSQ1[:, t : t + 1])
        NSQ2 = small.tile([128, NT], F32, tag="nsq2")
        for t in range(NT):
            s2 = sq.tile([128, D], F32, tag="sq2")
            nc.scalar.activation(out=s2[:], in_=A2[:, t, :], func=ACTF.Square,
                                 accum_out=NSQ2[:, t : t + 1])

        # ---- w = (iota < n) * 1/sqrt(nsq) ----
        RSQ1 = small.tile([128, NT], F32, tag="rsq1")
        nc.vector.reciprocal(RSQ1[:], NSQ1[:])
        WN1 = small.tile([128, NT], F32, tag="wn1")
        nc.scalar.activation(out=WN1[:], in_=RSQ1[:], func=ACTF.Sqrt)
        V1 = small.tile([128, NT], F32, tag="v1")
        nc.vector.tensor_scalar(out=V1[:], in0=IOTA[:],
                                scalar1=N1B[:, b : b + 1], scalar2=None,
                                op0=ALU.is_lt)
        W1 = small.tile([128, NT], F32, tag="w1")
        nc.vector.tensor_mul(W1[:], V1[:], WN1[:])
        RSQ2 = small.tile([128, NT], F32, tag="rsq2")
        nc.vector.reciprocal(RSQ2[:], NSQ2[:])
        WN2 = small.tile([128, NT], F32, tag="wn2")
        nc.scalar.activation(out=WN2[:], in_=RSQ2[:], func=ACTF.Sqrt)
        V2 = small.tile([128, NT], F32, tag="v2")
        nc.vector.tensor_scalar(out=V2[:], in0=IOTA[:],
                                scalar1=N2B[:, b : b + 1], scalar2=None,
                                op0=ALU.is_lt)
        W2 = small.tile([128, NT], F32, tag="w2")
        nc.vector.tensor_mul(W2[:], V2[:], WN2[:])

        # ---- scaled cast: C = bf16(A * w)  (7 on DVE, 1 on ACT) ----
        C1 = cast.tile([128, NT, D], BF16, tag="c1")
        for t in range(NT):
            nc.vector.tensor_scalar_mul(C1[:, t, :], A1[:, t, :],
                                        W1[:, t : t + 1])
        C2 = cast.tile([128, NT, D], BF16, tag="c2")
        for t in range(NT):
            if t == 0:
                nc.scalar.activation(out=C2[:, t, :], in_=A2[:, t, :],
                                     func=ACTF.Copy, scale=W2[:, t : t + 1])
            else:
                nc.vector.tensor_scalar_mul(C2[:, t, :], A2[:, t, :],
                                            W2[:, t : t + 1])

        # ---- marshal: T[k][d, 128t+p] = C[p, t, d]  (PE transpose, bf16;
        # two k-blocks share one PSUM bank, copied out in one op) ----
        T1 = tT.tile([128, NK, S], BF16, tag="t1")
        T2 = tT.tile([128, NK, S], BF16, tag="t2")
        for x, (C, T) in enumerate(((C1, T1), (C2, T2))):
            for kp in range(NK // 2):
                PT = psT.tile([128, 2, S], BF16, tag="pt")
                for h in range(2):
                    k = 2 * kp + h
                    for t in range(NT):
                        nc.tensor.transpose(
                            out=PT[:, h, 128 * t : 128 * (t + 1)],
                            in_=C[:, t, 128 * k : 128 * (k + 1)],
                            identity=IDB[:])
                if (x * 3 + kp) % 3 == 0:
                    nc.vector.tensor_copy(
                        T[:, 2 * kp : 2 * kp + 2, :].rearrange("p a b -> p (a b)"),
                        PT.rearrange("p a b -> p (a b)"))
                else:
                    nc.scalar.copy(
                        T[:, 2 * kp : 2 * kp + 2, :].rearrange("p a b -> p (a b)"),
                        PT.rearrange("p a b -> p (a b)"))

        # ---- main matmul: SIM [128, 4, 512] f32 (4 PSUM banks) ----
        SIM = psS.tile([128, NT, S], F32, tag="sim")
        for t in range(NT):
            for k in range(NK):
                nc.tensor.matmul(out=SIM[:, t, :],
                                 lhsT=T1[:, k, 128 * t : 128 * (t + 1)],
                                 rhs=T2[:, k, :], start=(k == 0),
                                 stop=(k == NK - 1))

        # ---- row max (one 3D reduce), fold, transpose, col max ----
        nc.vector.tensor_reduce(out=RC[:, 8 * b : 8 * b + NT], in_=SIM[:],
                                axis=AX.X, op=ALU.max)
        FT = fold.tile([128, S], BF16, tag="ft")
        nc.vector.tensor_reduce(out=FT[:], in_=SIM.rearrange("p t j -> p j t"),
                                axis=AX.X, op=ALU.max)
        PF = psF.tile([128, NT, 128], BF16, tag="pf")
        for t in range(NT):
            nc.tensor.transpose(out=PF[:, t, :],
                                in_=FT[:, 128 * t : 128 * (t + 1)],
                                identity=IDB[:])
        nc.vector.tensor_reduce(out=RC[:, 8 * b + NT : 8 * b + 8], in_=PF[:],
                                axis=AX.X, op=ALU.max)

    # ---- epilogue: scores = (sum over RC columns, grouped by batch) / (n1+n2) ----
    pe_sum = psE.tile([128, 64], F32, tag="eps")
    nc.tensor.matmul(out=pe_sum[:1, : BPC * 8], lhsT=ONES[:], rhs=RC[:],
                     start=True, stop=True)
    SC = consts.tile([1, BPC * 8], F32, tag="sc")
    nc.vector.tensor_copy(SC[:], pe_sum[:1, : BPC * 8])
    TOT = consts.tile([1, BPC], F32, tag="tot")
    nc.vector.tensor_reduce(out=TOT[:], in_=SC.rearrange("p (g x) -> p g x", x=8),
                            axis=AX.X, op=ALU.add)
    pe_t = psE.tile([128, 64], F32, tag="eps")
    nc.tensor.matmul(out=pe_t[:BPC, :1], lhsT=TOT[:], rhs=ONES[:1, :1],
                     start=True, stop=True)
    SCT = consts.tile([BPC, 1], F32, tag="sct")
    nc.vector.tensor_copy(SCT[:], pe_t[:BPC, :1])
    OUT = consts.tile([BPC, 1], F32, tag="out")
    nc.vector.tensor_mul(OUT[:], SCT[:], rns[:])
    nc.sync.dma_start(out=aps["out"][:], in_=OUT[:])


_CACHE = {}


def _build():
    if "nc" in _CACHE:
        return _CACHE["nc"]
    nc = bacc.Bacc("TRN2", target_bir_lowering=False, debug=False,
                   num_devices=N_CORES)
    aps = {
        "a1": nc.dram_tensor("a1", [BPC, S, D], F32, kind="ExternalInput").ap(),
        "a2": nc.dram_tensor("a2", [BPC, S, D], F32, kind="ExternalInput").ap(),
        "m1": nc.dram_tensor("m1", [BPC, S], I32, kind="ExternalInput").ap(),
        "m2": nc.dram_tensor("m2", [BPC, S], I32, kind="ExternalInput").ap(),
        "idb": nc.dram_tensor("idb", [128, 128], BF16, kind="ExternalInput").ap(),
        "idf": nc.dram_tensor("idf", [128, 128], F32, kind="ExternalInput").ap(),
        "iota": nc.dram_tensor("iota", [128, NT], F32, kind="ExternalInput").ap(),
        "out": nc.dram_tensor("out", [BPC, 1], F32, kind="ExternalOutput").ap(),
    }
    with tile.TileContext(nc) as tc:
        _emit(tc, aps)
    nc.compile()
    _CACHE["nc"] = nc
    return nc


def _consts():
    return {
        "idb": np.eye(128, dtype=ml_dtypes.bfloat16),
        "idf": np.eye(128, dtype=np.float32),
        # token(p, t) = 4p + t  (flat contiguous DMA mapping)
        "iota": (4.0 * np.arange(128, dtype=np.float32)[:, None]
                 + np.arange(NT, dtype=np.float32)[None, :]),
    }


def make_in_maps(article_1_emb, article_2_emb, article_1_att_mask,
                 article_2_att_mask):
    a1 = np.ascontiguousarray(np.asarray(article_1_emb, dtype=np.float32))
    a2 = np.ascontiguousarray(np.asarray(article_2_emb, dtype=np.float32))
    m1 = np.ascontiguousarray(np.asarray(article_1_att_mask, dtype=np.int32))
    m2 = np.ascontiguousarray(np.asarray(article_2_att_mask, dtype=np.int32))
    cst = _consts()
    in_maps = []
    for c in range(N_CORES):
        sl = slice(c * BPC, (c + 1) * BPC)
        in_maps.append({"a1": a1[sl], "a2": a2[sl], "m1": m1[sl], "m2": m2[sl],
                        **cst})
    return in_maps


def _ensure_profile_hook():
    """bass_utils' axon trace path imports antenv.axon_hooks, which this
    image lacks. Inject it and register the ctypes NTFF hook."""
    import types

    if "antenv.axon_hooks" in sys.modules:
        return
    mod = types.ModuleType("antenv.axon_hooks")
    mod._hook = None
    mod.set_axon_ntff_profile_hook = lambda h: setattr(mod, "_hook", h)
    mod.get_axon_ntff_profile_hook = lambda: mod._hook
    sys.modules["antenv.axon_hooks"] = mod
    try:
        from trn_agent_boot.trn_boot import _ntff_profile_via_ctypes
        mod._hook = _ntff_profile_via_ctypes("/opt/axon/libaxon_pjrt.so")
    except Exception as e:
        print("ntff hook setup failed:", e)


def kernel(article_1_emb, article_2_emb, article_1_att_mask,
           article_2_att_mask, _trace=False, _trace_kwargs=None):
    if _trace:
        _ensure_profile_hook()
    nc = _build()
    in_maps = make_in_maps(article_1_emb, article_2_emb, article_1_att_mask,
                           article_2_att_mask)
    res = bass_utils.run_bass_kernel_spmd(
        nc, in_maps, core_ids=list(range(N_CORES)), trace=_trace,
        **(_trace_kwargs or {}))
    out = np.concatenate([np.asarray(res.results[c]["out"]).reshape(BPC)
                          for c in range(N_CORES)])
    if _trace:
        return out.astype(np.float32), res
    return out.astype(np.float32)


if __name__ == "__main__":
    # quick CoreSim check against numpy for core 0's slice
    rng = np.random.default_rng(0)
    a1 = rng.standard_normal((BPC, S, D), dtype=np.float32)
    a2 = rng.standard_normal((BPC, S, D), dtype=np.float32)
    m1 = rng.integers(0, 2, size=(BPC, S)).astype(np.int32)
    m2 = rng.integers(0, 2, size=(BPC, S)).astype(np.int32)

    nc = _build()
    print("compiled ok", flush=True)

    from concourse.bass_interp import CoreSim
    sim = CoreSim(nc)
    cst = _consts()
    for k, v in (("a1", a1), ("a2", a2), ("m1", m1), ("m2", m2), *cst.items()):
        sim.tensor(k)[:] = v
    sim.simulate()
    got = np.asarray(sim.tensor("out")).reshape(BPC)

    n1 = m1.sum(-1); n2 = m2.sum(-1)
    pos = np.arange(S)
    w1 = (pos[None, :] < n1[:, None]) / np.linalg.norm(a1, axis=-1)
    w2 = (pos[None, :] < n2[:, None]) / np.linalg.norm(a2, axis=-1)
    M = np.einsum("bid,bjd->bij", a1 * w1[..., None], a2 * w2[..., None])
    want = (M.max(2).sum(-1) + M.max(1).sum(-1)) / (n1 + n2)
    print("sim:", got)
    print("ref:", want)
    print("rel err:", np.abs(got - want).max() / np.abs(want).max())


# revision 11
# speedup vs baseline: 1.3635x; 1.0845x over previous
"""Trainium2 Bass kernel for nn_ContrastiveModel (ColBERT-style MaxSim scoring).

score[b] = (sum_i max_j cos(a1[b,i], a2[b,j]) + sum_j max_i cos(...)) / (n1+n2)
with prefix validity masks (pos < sum(att_mask)).

Strategy (pure data parallel, 8 batches per core):
  - flat contiguous DMA: SBUF [128, 4, 768], token(p,t) = 4p+t (reductions are
    permutation invariant; the iota constant encodes the same mapping)
  - w[s] = valid(s) / ||a[s]||; norms via ACT square+accum, w via DVE
    reciprocal + ACT sqrt + iota compare
  - normalization+masking fused into the f32->bf16 cast (tensor_scalar mult
    by per-partition w, one op per token-block)
  - transpose to [d, token] layout via PE transpose-mode (bf16 PSUM out),
    copied to SBUF by DVE/ACT
  - main matmul: SIM[128i, 4, 512j] (4 PSUM banks) = T1^T @ T2 (bf16)
  - row max / fold / col max as single 3D strided DVE reduces
  - invalid rows/cols are exactly zero => contribute 0 (max over valid
    entries is > 0 for this data, verified vs reference)
  - epilogue: ones-matmul partition sums, scores = total * 1/(n1+n2)
"""

import sys

sys.path.insert(0, "/opt/trn_rl_repo")

import numpy as np
import ml_dtypes
from contextlib import ExitStack

import concourse.bacc as bacc
import concourse.bass as bass
import concourse.tile as tile
from concourse import mybir
from concourse import bass_utils
from concourse._compat import with_exitstack


def _axon_device_reset():
    """The terminal-side accelerator is often left unrecoverable by a previous
    client process; axon_reset clears that state. Call before first use."""
    import ctypes
    try:
        lib = ctypes.CDLL("/opt/axon/libaxon_pjrt.so")
        lib.axon_reset.restype = ctypes.c_int64
        rc = lib.axon_reset()
        if rc != 0:
            print("axon_reset rc:", rc)
    except Exception as e:
        print("axon_reset failed:", e)


_axon_device_reset()

N_CORES = 8
B_FULL, S, D = 64, 512, 768
BPC = B_FULL // N_CORES  # batches per core
NT = S // 128  # token blocks (4)
NK = D // 128  # d blocks (6)

F32 = mybir.dt.float32
BF16 = mybir.dt.bfloat16
I32 = mybir.dt.int32
AX = mybir.AxisListType
ALU = mybir.AluOpType
ACTF = mybir.ActivationFunctionType


@with_exitstack
def _emit(ctx: ExitStack, tc: tile.TileContext, aps: dict):
    nc = tc.nc

    # flat contiguous load: partition p holds tokens 4p..4p+3
    a1r = aps["a1"].rearrange("b (p x) d -> b p (x d)", p=128)
    a2r = aps["a2"].rearrange("b (p x) d -> b p (x d)", p=128)

    consts = ctx.enter_context(tc.tile_pool(name="consts", bufs=1))
    araw = ctx.enter_context(tc.tile_pool(name="araw", bufs=3))
    sq = ctx.enter_context(tc.tile_pool(name="sq", bufs=2))
    small = ctx.enter_context(tc.tile_pool(name="small", bufs=2))
    cast = ctx.enter_context(tc.tile_pool(name="cast", bufs=2))
    tT = ctx.enter_context(tc.tile_pool(name="tT", bufs=2))
    fold = ctx.enter_context(tc.tile_pool(name="fold", bufs=2))
    psT = ctx.enter_context(tc.tile_pool(name="psT", bufs=2, space="PSUM"))
    psT2 = ctx.enter_context(tc.tile_pool(name="psT2", bufs=1, space="PSUM"))
    psS = ctx.enter_context(tc.tile_pool(name="psS", bufs=1, space="PSUM"))
    psE = ctx.enter_context(tc.tile_pool(name="psE", bufs=1, space="PSUM"))

    # ---- constants ----
    IDB = consts.tile([128, 128], BF16, tag="idb")
    nc.sync.dma_start(out=IDB[:], in_=aps["idb"][:])
    IDF = consts.tile([128, 128], F32, tag="idf")
    nc.sync.dma_start(out=IDF[:], in_=aps["idf"][:])
    IOTA = consts.tile([128, NT], F32, tag="iota")
    nc.sync.dma_start(out=IOTA[:], in_=aps["iota"][:])
    ONES = consts.tile([128, 1], F32, tag="ones")
    nc.vector.memset(ONES[:], 1.0)

    # ---- masks -> n1, n2, 1/(n1+n2), broadcast n to all partitions ----
    M1i = consts.tile([BPC, S], I32, tag="m1i")
    nc.sync.dma_start(out=M1i[:], in_=aps["m1"][:])
    M2i = consts.tile([BPC, S], I32, tag="m2i")
    nc.sync.dma_start(out=M2i[:], in_=aps["m2"][:])
    M1f = consts.tile([BPC, S], F32, tag="m1f")
    nc.vector.tensor_copy(M1f[:], M1i[:])
    M2f = consts.tile([BPC, S], F32, tag="m2f")
    nc.vector.tensor_copy(M2f[:], M2i[:])
    n1 = consts.tile([BPC, 1], F32, tag="n1")
    nc.vector.tensor_reduce(out=n1[:], in_=M1f[:], axis=AX.X, op=ALU.add)
    n2 = consts.tile([BPC, 1], F32, tag="n2")
    nc.vector.tensor_reduce(out=n2[:], in_=M2f[:], axis=AX.X, op=ALU.add)
    ns = consts.tile([BPC, 1], F32, tag="ns")
    nc.vector.tensor_add(ns[:], n1[:], n2[:])
    rns = consts.tile([BPC, 1], F32, tag="rns")
    nc.vector.reciprocal(rns[:], ns[:])

    # spread n1/n2 to the free dim: [BPC,1] -> [1,BPC] via PE, then broadcast
    pe_n1 = psE.tile([128, 64], F32, tag="eps")
    nc.tensor.matmul(out=pe_n1[:1, :BPC], lhsT=n1[:], rhs=IDF[:BPC, :BPC],
                     start=True, stop=True)
    N1r = consts.tile([1, BPC], F32, tag="n1r")
    nc.vector.tensor_copy(N1r[:], pe_n1[:1, :BPC])
    pe_n2 = psE.tile([128, 64], F32, tag="eps")
    nc.tensor.matmul(out=pe_n2[:1, :BPC], lhsT=n2[:], rhs=IDF[:BPC, :BPC],
                     start=True, stop=True)
    N2r = consts.tile([1, BPC], F32, tag="n2r")
    nc.vector.tensor_copy(N2r[:], pe_n2[:1, :BPC])
    N1B = consts.tile([128, BPC], F32, tag="n1b")
    nc.gpsimd.partition_broadcast(N1B[:], N1r[:])
    N2B = consts.tile([128, BPC], F32, tag="n2b")
    nc.gpsimd.partition_broadcast(N2B[:], N2r[:])

    # result collector: 8 columns per batch (4 row-max blocks + 4 col-max blocks)
    RC = consts.tile([128, BPC * 8], F32, tag="rc")

    for b in range(BPC):
        # ---- load (split across the two HWDGE rings) ----
        A1 = araw.tile([128, NT, D], F32, tag="a1")
        nc.sync.dma_start(out=A1.rearrange("p t d -> p (t d)"), in_=a1r[b])
        A2 = araw.tile([128, NT, D], F32, tag="a2")
        nc.scalar.dma_start(out=A2.rearrange("p t d -> p (t d)"), in_=a2r[b])

        # ---- norms^2 (ACT square + accumulate) ----
        NSQ1 = small.tile([128, NT], F32, tag="nsq1")
        for t in range(NT):
            s1 = sq.tile([128, D], F32, tag="sq1")
            nc.scalar.activation(out=s1[:], in_=A1[:, t, :], func=ACTF.Square,
                                 accum_out=NSQ1[:, t : t + 1])
        NSQ2 = small.tile([128, NT], F32, tag="nsq2")
        for t in range(NT):
            s2 = sq.tile([128, D], F32, tag="sq2")
            nc.scalar.activation(out=s2[:], in_=A2[:, t, :], func=ACTF.Square,
                                 accum_out=NSQ2[:, t : t + 1])

        # ---- w = (iota < n) * 1/sqrt(nsq) ----
        RSQ1 = small.tile([128, NT], F32, tag="rsq1")
        nc.vector.reciprocal(RSQ1[:], NSQ1[:])
        WN1 = small.tile([128, NT], F32, tag="wn1")
        nc.scalar.activation(out=WN1[:], in_=RSQ1[:], func=ACTF.Sqrt)
        V1 = small.tile([128, NT], F32, tag="v1")
        nc.vector.tensor_scalar(out=V1[:], in0=IOTA[:],
                                scalar1=N1B[:, b : b + 1], scalar2=None,
                                op0=ALU.is_lt)
        W1 = small.tile([128, NT], F32, tag="w1")
        nc.vector.tensor_mul(W1[:], V1[:], WN1[:])
        RSQ2 = small.tile([128, NT], F32, tag="rsq2")
        nc.vector.reciprocal(RSQ2[:], NSQ2[:])
        WN2 = small.tile([128, NT], F32, tag="wn2")
        nc.scalar.activation(out=WN2[:], in_=RSQ2[:], func=ACTF.Sqrt)
        V2 = small.tile([128, NT], F32, tag="v2")
        nc.vector.tensor_scalar(out=V2[:], in0=IOTA[:],
                                scalar1=N2B[:, b : b + 1], scalar2=None,
                                op0=ALU.is_lt)
        W2 = small.tile([128, NT], F32, tag="w2")
        nc.vector.tensor_mul(W2[:], V2[:], WN2[:])

        # ---- scaled cast: C = bf16(A * w)  (7 on DVE, 1 on ACT) ----
        C1 = cast.tile([128, NT, D], BF16, tag="c1")
        for t in range(NT):
            nc.vector.tensor_scalar_mul(C1[:, t, :], A1[:, t, :],
                                        W1[:, t : t + 1])
        C2 = cast.tile([128, NT, D], BF16, tag="c2")
        for t in range(NT):
            if t == 0:
                nc.scalar.activation(out=C2[:, t, :], in_=A2[:, t, :],
                                     func=ACTF.Copy, scale=W2[:, t : t + 1])
            else:
                nc.vector.tensor_scalar_mul(C2[:, t, :], A2[:, t, :],
                                            W2[:, t : t + 1])

        # ---- marshal: T[k][d, 128t+p] = C[p, t, d] ----
        # a1 via REGULAR matmuls against identity (counts as PE activity so
        # the HAM clock gate stays at 2.4 GHz; transpose-mode ops do not).
        # a2 via transpose-mode (bf16 PSUM -> half-cost copies).
        T1 = tT.tile([128, NK, S], BF16, tag="t1")
        for k in range(NK):
            PT = psT.tile([128, S], F32, tag="pt")
            for t in range(NT):
                nc.tensor.matmul(out=PT[:, 128 * t : 128 * (t + 1)],
                                 lhsT=C1[:, t, 128 * k : 128 * (k + 1)],
                                 rhs=IDB[:], start=True, stop=True)
            if k % 2 == 0:
                nc.vector.tensor_copy(T1[:, k, :], PT[:])
            else:
                nc.scalar.copy(T1[:, k, :], PT[:])
        T2 = tT.tile([128, NK, S], BF16, tag="t2")
        for kp in range(NK // 2):
            PT2 = psT2.tile([128, 2, S], BF16, tag="pt2")
            for h in range(2):
                k = 2 * kp + h
                for t in range(NT):
                    nc.tensor.transpose(
                        out=PT2[:, h, 128 * t : 128 * (t + 1)],
                        in_=C2[:, t, 128 * k : 128 * (k + 1)],
                        identity=IDB[:])
            if kp % 3 == 0:
                nc.scalar.copy(
                    T2[:, 2 * kp : 2 * kp + 2, :].rearrange("p a b -> p (a b)"),
                    PT2.rearrange("p a b -> p (a b)"))
            else:
                nc.vector.tensor_copy(
                    T2[:, 2 * kp : 2 * kp + 2, :].rearrange("p a b -> p (a b)"),
                    PT2.rearrange("p a b -> p (a b)"))

        # ---- main matmul: SIM [128, 4, 512] f32 (4 PSUM banks) ----
        SIM = psS.tile([128, NT, S], F32, tag="sim")
        for t in range(NT):
            for k in range(NK):
                nc.tensor.matmul(out=SIM[:, t, :],
                                 lhsT=T1[:, k, 128 * t : 128 * (t + 1)],
                                 rhs=T2[:, k, :], start=(k == 0),
                                 stop=(k == NK - 1))

        # ---- row max (one 3D reduce), fold, transpose, col max ----
        nc.vector.tensor_reduce(out=RC[:, 8 * b : 8 * b + NT], in_=SIM[:],
                                axis=AX.X, op=ALU.max)
        FT = fold.tile([128, S], BF16, tag="ft")
        nc.vector.tensor_reduce(out=FT[:], in_=SIM.rearrange("p t j -> p j t"),
                                axis=AX.X, op=ALU.max)
        PF = psE.tile([128, NT, 128], F32, tag="eps")
        for t in range(NT):
            nc.tensor.matmul(out=PF[:, t, :],
                             lhsT=FT[:, 128 * t : 128 * (t + 1)],
                             rhs=IDB[:], start=True, stop=True)
        nc.vector.tensor_reduce(out=RC[:, 8 * b + NT : 8 * b + 8], in_=PF[:],
                                axis=AX.X, op=ALU.max)

    # ---- epilogue: scores = (sum over RC columns, grouped by batch) / (n1+n2) ----
    pe_sum = psE.tile([128, 64], F32, tag="eps")
    nc.tensor.matmul(out=pe_sum[:1, : BPC * 8], lhsT=ONES[:], rhs=RC[:],
                     start=True, stop=True)
    SC = consts.tile([1, BPC * 8], F32, tag="sc")
    nc.vector.tensor_copy(SC[:], pe_sum[:1, : BPC * 8])
    TOT = consts.tile([1, BPC], F32, tag="tot")
    nc.vector.tensor_reduce(out=TOT[:], in_=SC.rearrange("p (g x) -> p g x", x=8),
                            axis=AX.X, op=ALU.add)
    pe_t = psE.tile([128, 64], F32, tag="eps")
    nc.tensor.matmul(out=pe_t[:BPC, :1], lhsT=TOT[:], rhs=ONES[:1, :1],
                     start=True, stop=True)
    SCT = consts.tile([BPC, 1], F32, tag="sct")
    nc.vector.tensor_copy(SCT[:], pe_t[:BPC, :1])
    OUT = consts.tile([BPC, 1], F32, tag="out")
    nc.vector.tensor_mul(OUT[:], SCT[:], rns[:])
    nc.sync.dma_start(out=aps["out"][:], in_=OUT[:])


_CACHE = {}


def _build():
    if "nc" in _CACHE:
        return _CACHE["nc"]
    nc = bacc.Bacc("TRN2", target_bir_lowering=False, debug=False,
                   num_devices=N_CORES)
    aps = {
        "a1": nc.dram_tensor("a1", [BPC, S, D], F32, kind="ExternalInput").ap(),
        "a2": nc.dram_tensor("a2", [BPC, S, D], F32, kind="ExternalInput").ap(),
        "m1": nc.dram_tensor("m1", [BPC, S], I32, kind="ExternalInput").ap(),
        "m2": nc.dram_tensor("m2", [BPC, S], I32, kind="ExternalInput").ap(),
        "idb": nc.dram_tensor("idb", [128, 128], BF16, kind="ExternalInput").ap(),
        "idf": nc.dram_tensor("idf", [128, 128], F32, kind="ExternalInput").ap(),
        "iota": nc.dram_tensor("iota", [128, NT], F32, kind="ExternalInput").ap(),
        "out": nc.dram_tensor("out", [BPC, 1], F32, kind="ExternalOutput").ap(),
    }
    with tile.TileContext(nc) as tc:
        _emit(tc, aps)
    nc.compile()
    _CACHE["nc"] = nc
    return nc


def _consts():
    return {
        "idb": np.eye(128, dtype=ml_dtypes.bfloat16),
        "idf": np.eye(128, dtype=np.float32),
        # token(p, t) = 4p + t  (flat contiguous DMA mapping)
        "iota": (4.0 * np.arange(128, dtype=np.float32)[:, None]
                 + np.arange(NT, dtype=np.float32)[None, :]),
    }


def make_in_maps(article_1_emb, article_2_emb, article_1_att_mask,
                 article_2_att_mask):
    a1 = np.ascontiguousarray(np.asarray(article_1_emb, dtype=np.float32))
    a2 = np.ascontiguousarray(np.asarray(article_2_emb, dtype=np.float32))
    m1 = np.ascontiguousarray(np.asarray(article_1_att_mask, dtype=np.int32))
    m2 = np.ascontiguousarray(np.asarray(article_2_att_mask, dtype=np.int32))
    cst = _consts()
    in_maps = []
    for c in range(N_CORES):
        sl = slice(c * BPC, (c + 1) * BPC)
        in_maps.append({"a1": a1[sl], "a2": a2[sl], "m1": m1[sl], "m2": m2[sl],
                        **cst})
    return in_maps


def _ensure_profile_hook():
    """bass_utils' axon trace path imports antenv.axon_hooks, which this
    image lacks. Inject it and register the ctypes NTFF hook."""
    import types

    if "antenv.axon_hooks" in sys.modules:
        return
    mod = types.ModuleType("antenv.axon_hooks")
    mod._hook = None
    mod.set_axon_ntff_profile_hook = lambda h: setattr(mod, "_hook", h)
    mod.get_axon_ntff_profile_hook = lambda: mod._hook
    sys.modules["antenv.axon_hooks"] = mod
    try:
        from trn_agent_boot.trn_boot import _ntff_profile_via_ctypes
        mod._hook = _ntff_profile_via_ctypes("/opt/axon/libaxon_pjrt.so")
    except Exception as e:
        print("ntff hook setup failed:", e)


def kernel(article_1_emb, article_2_emb, article_1_att_mask,
           article_2_att_mask, _trace=False, _trace_kwargs=None):
    if _trace:
        _ensure_profile_hook()
    nc = _build()
    in_maps = make_in_maps(article_1_emb, article_2_emb, article_1_att_mask,
                           article_2_att_mask)
    res = bass_utils.run_bass_kernel_spmd(
        nc, in_maps, core_ids=list(range(N_CORES)), trace=_trace,
        **(_trace_kwargs or {}))
    out = np.concatenate([np.asarray(res.results[c]["out"]).reshape(BPC)
                          for c in range(N_CORES)])
    if _trace:
        return out.astype(np.float32), res
    return out.astype(np.float32)


if __name__ == "__main__":
    # quick CoreSim check against numpy for core 0's slice
    rng = np.random.default_rng(0)
    a1 = rng.standard_normal((BPC, S, D), dtype=np.float32)
    a2 = rng.standard_normal((BPC, S, D), dtype=np.float32)
    m1 = rng.integers(0, 2, size=(BPC, S)).astype(np.int32)
    m2 = rng.integers(0, 2, size=(BPC, S)).astype(np.int32)

    nc = _build()
    print("compiled ok", flush=True)

    from concourse.bass_interp import CoreSim
    sim = CoreSim(nc)
    cst = _consts()
    for k, v in (("a1", a1), ("a2", a2), ("m1", m1), ("m2", m2), *cst.items()):
        sim.tensor(k)[:] = v
    sim.simulate()
    got = np.asarray(sim.tensor("out")).reshape(BPC)

    n1 = m1.sum(-1); n2 = m2.sum(-1)
    pos = np.arange(S)
    w1 = (pos[None, :] < n1[:, None]) / np.linalg.norm(a1, axis=-1)
    w2 = (pos[None, :] < n2[:, None]) / np.linalg.norm(a2, axis=-1)
    M = np.einsum("bid,bjd->bij", a1 * w1[..., None], a2 * w2[..., None])
    want = (M.max(2).sum(-1) + M.max(1).sum(-1)) / (n1 + n2)
    print("sim:", got)
    print("ref:", want)
    print("rel err:", np.abs(got - want).max() / np.abs(want).max())
